# revision 66
# baseline (speedup 1.0000x reference)
"""Self-contained MHA kernel for Trainium2, 8 NeuronCores — v4.

Problem: B=4, T=2048, D=1024, H=16 causal MHA, fp32, no bias.
Sharding: core c handles batch b=c//2 and head-group hg=c%2 (8 heads = 4
head-pairs), Megatron-style: Wq/Wk/Wv column-sharded, Wo row-sharded; host
sums the two partial outputs per batch (and divides by the 32x weight
pre-scale).

Design notes:
 - projections and out-projection run as fp8e4m3 hi/lo 3-term DoubleRow
   matmuls (2 k-tiles per instruction at 0.5 cyc/row): x = xh+xl, W' = 32W
   = wh+wl (the 32x pre-scale keeps the lo residuals out of fp8 subnormals;
   the scale is undone via the exp scale, the ones column, and a host-side
   divide). Terms: xh*wh + xl*wh + xh*wl.
 - scores S^T[k,q] in bf16 (2 heads via PE quadrants), exp on ACT with
   scale 2^-13 (= 1/8 softmax scale / 32^2), no max subtraction
 - ctx: se tile [k,q] is the STATIONARY, [V|32] the moving operand ->
   ctx[q, dv+1] accumulated over k-tiles, 65 moving rows per (ktile, qtile,
   head), fused denominator column; groups run strictly sequentially per
   PSUM bank (start=True marks the whole 2KB bank pending-zero)
 - V projected directly into [t, dv] layout (x-tile stationary)
 - ctx normalized on DVE (reciprocal + broadcast multiply), transposed
   [q,dv]->[dv,q] by the DMA xbar, then split to fp8 hi/lo for the
   out-projection

v4 scheduling (ACT is the pacing engine; the PE must never emit work
ahead of the score stream that ACT's exps depend on):
 - preroll: each phase's first score k-tiles are emitted BEFORE the
   previous chunk's ctx drain, so ACT always has fresh psum tiles while
   the PE chews the ctx block (-4us)
 - V projections carry over pair boundaries as fillers (only Q/K flush
   at the boundary), with force_v draining them just before the ctx
   that consumes them
 - pctx is two single-bank [128, 2, 2, VW] psum tiles (qtile pairs
   j01/j23), so a half-chunk normalize never WAR-blocks the other half
 - the final chunk's ctx groups interleave into its own QK stream
   (group j at ki>=13+j) with per-half normalize, cascading the ctxT
   chains so the out-projection tail starts as early as possible
 - out-store DMAs are deferred behind each drain's ctxT transpose on
   the shared SP/HWDGE queue
"""

import os

import numpy as np

B, T, D, H = 4, 2048, 1024, 16
SCHED_EVERY = 4   # filler pop cadence (1/N of ki slots)
SCHED_EARLY = 1   # start pair-0 attention after chunk-0 proj only
SCHED_CTXI = 0    # ctx groups run as a block after the next QK phase
SCHED_JPOP = 0
SCHED_P3 = 0
SCHED_WARM = 40   # PE p-state warmup matmuls during the input DMA
SCHED_ROWS = "0123"
SCHED_P3Q = 0     # per-qtile normalize chains for the last pair
SCHED_DEFV = 0    # defer last pair's V projection into its own phases
SCHED_ECDMA = 0   # store each 512-col out half as its own DMA
SCHED_WMEM = 1    # warmup source memset on Pool (starts earlier than DVE)
SCHED_EVERY3 = 3  # filler pop cadence in the last pair's phases
SCHED_CHDVE = 0   # ctxT hi/lo split on DVE instead of Pool
SCHED_HOLDV3 = 0  # stash last pair's V units for its own phase windows
SCHED_DRPOP = 0   # interleave ready fillers into ctx drains (every N)
SCHED_TSPLIT = 0  # split the last chunk's normalize into two halves
SCHED_PREROLL = 3
SCHED_CXSPLIT = 1 # separate single-bank psum tiles for ctx qtile pairs
SCHED_STQ = 1     # defer out-store DMAs to after the next ctx drain
SCHED_TACT = 0    # last chunk's ctxT transpose via the ACT hwdge queue
SCHED_FINEC = 0   # closing tiles with split evac + per-half stores
SCHED_STACT = 0   # route closing stores via the ACT hwdge queue
SCHED_GRPM = 0    # single psum group bracket per ctx bank
SCHED_LSPL = 0    # split the very last evac across ACT and DVE
SCHED_TRIP = 0    # last chunk's diagonal trimasks on Pool instead of DVE
SCHED_EVAC = 1    # 0: ACT evacs for qc>=2, 1: only final batch, 2: never
SCHED_ALT3 = 0    # borrow score psum for outproj only in the final batch
SCHED_CTXL = 13   # interleave the final chunk's ctx groups at ki >= this
SCHED_CTXLQ = 1   # per-qtile normalize inside the final chunk's interleave
DK = 64
NCORES = 8
NPAIR = 4        # head-pairs per core
ESH = 512        # output-feature shard per core (8 heads * 64)
VW = 65          # dv + 1 ones column (denominator)
WSCALE = 32.0    # weight pre-scale (keeps fp8 lo parts out of subnormals)

_nc_cache = None
MM_LABELS = []       # per-emitted-matmul debug labels (program order)
_CUR_LBL = ["?"]


def _lbl(s):
    _CUR_LBL[0] = s


def _build():
    global _nc_cache
    if _nc_cache is not None:
        return _nc_cache

    from contextlib import ExitStack

    import concourse.bacc as bacc
    import concourse.mybir as mybir
    import concourse.tile as tile
    from concourse.masks import make_upper_triangular

    F32 = mybir.dt.float32
    BF16 = mybir.dt.bfloat16
    FP8 = mybir.dt.float8e4
    AF = mybir.ActivationFunctionType
    OP = mybir.AluOpType
    DR = mybir.MatmulPerfMode.DoubleRow

    nc = bacc.Bacc("TRN2", target_bir_lowering=False, debug=False,
                   num_devices=NCORES)
    MM_LABELS.clear()
    _mm0 = nc.tensor.matmul

    def _mm(*a, **k):
        MM_LABELS.append(_CUR_LBL[0])
        return _mm0(*a, **k)
    nc.tensor.matmul = _mm
    xh_d = nc.declare_dram_parameter("xh", [D, T], FP8, isOutput=False)
    xl_d = nc.declare_dram_parameter("xl", [D, T], FP8, isOutput=False)
    w_ds = {}
    for wn in ("qh", "ql", "kh", "kl", "vh", "vl"):
        w_ds[wn] = nc.declare_dram_parameter(
            f"w{wn}", [NPAIR, 128, 8, 128], FP8, isOutput=False)
    wall_d = nc.declare_dram_parameter("wall", [NPAIR, 6, 128, 8, 128],
                                       FP8, isOutput=False)
    woh_d = nc.declare_dram_parameter("woh", [128, NPAIR, D], FP8,
                                      isOutput=False)
    wol_d = nc.declare_dram_parameter("wol", [128, NPAIR, D], FP8,
                                      isOutput=False)
    out_d = nc.declare_dram_parameter("out", [T, D], BF16, isOutput=True)

    with tile.TileContext(nc) as tc, ExitStack() as ctx:
        const_p = ctx.enter_context(tc.tile_pool(name="const", bufs=1))
        xt_p = ctx.enter_context(tc.tile_pool(name="xt", bufs=2))
        w_p = ctx.enter_context(tc.tile_pool(name="w", bufs=6))
        wall_p = ctx.enter_context(tc.tile_pool(name="wall", bufs=3))
        wo_p = ctx.enter_context(tc.tile_pool(name="wo", bufs=2))
        qk_p = ctx.enter_context(tc.tile_pool(name="qk", bufs=6))
        v_p = ctx.enter_context(tc.tile_pool(name="v", bufs=4))
        se_p = ctx.enter_context(tc.tile_pool(name="se", bufs=28))
        cu_p = ctx.enter_context(tc.tile_pool(name="cu", bufs=2))
        cb_p = ctx.enter_context(tc.tile_pool(name="cb", bufs=3))
        rec_p = ctx.enter_context(tc.tile_pool(name="rec", bufs=2))
        ctxT_p = ctx.enter_context(tc.tile_pool(name="ctxT", bufs=1))
        st_p = ctx.enter_context(tc.tile_pool(name="st", bufs=8))
        psum_pr = ctx.enter_context(tc.tile_pool(name="psum_pr", bufs=2,
                                                 space="PSUM"))
        psum_ss = ctx.enter_context(tc.tile_pool(name="psum_ss", bufs=2,
                                                 space="PSUM"))
        psum_cx = ctx.enter_context(tc.tile_pool(name="psum_cx", bufs=1,
                                                 space="PSUM"))

        # warmup source first: its memset must head the Pool queue so the
        # p-state warmup matmuls start as early as possible
        wsrc = cb_p.tile([128, 128], BF16, tag="warm", name="warmsrc")
        if SCHED_WARM:
            if SCHED_WMEM:
                nc.gpsimd.memset(wsrc[:], 1.0)
            else:
                nc.vector.memset(wsrc[:], 1.0)

        # constants
        trimask = const_p.tile([128, 128], BF16)
        make_upper_triangular(nc, trimask[:], val=1.0, diag=True)

        # persistent tensors
        xh = xt_p.tile([128, 8, T], FP8, tag="xt", name="xh")
        xl = xt_p.tile([128, 8, T], FP8, tag="xt", name="xl")
        ctxTh = ctxT_p.tile([128, NPAIR, T], FP8, tag="ctxTh", name="ctxTh")
        ctxTl = ctxT_p.tile([128, NPAIR, T], FP8, tag="ctxTl", name="ctxTl")

        W_TILES = {}   # p -> dict of 6 hi/lo weight tiles
        ST_TILES = {}
        STQ = []       # deferred out-store DMA emitters
        QKV = {}       # p -> (QT, KT, V)
        WO = {}
        PROJ_PS = {}

        def emit_x_dma(c):
            csl = slice(c * 512, (c + 1) * 512)
            for t_sb, t_d in ((xh, xh_d), (xl, xl_d)):
                nc.sync.dma_start(
                    out=t_sb[:, :, csl],
                    in_=t_d[:, csl].rearrange("(a p) t -> p a t", p=128))

        WNAMES = ("qh", "ql", "kh", "kl", "vh", "vl")

        def emit_w_dma(p):
            # one DMA for the whole pair's 6 hi/lo weight tensors
            wall = wall_p.tile([128, 6, 8, 128], FP8, tag="wall",
                               name=f"wall{p}")
            nc.sync.dma_start(out=wall[:],
                              in_=wall_d[p].rearrange("w p a f -> p w a f"))
            W_TILES[p] = {wn: wall[:, i] for i, wn in enumerate(WNAMES)}

        def emit_x_dma_half(c, which):
            csl = slice(c * 512, (c + 1) * 512)
            t_sb, t_d = (xh, xh_d) if which == "h" else (xl, xl_d)
            nc.sync.dma_start(
                out=t_sb[:, :, csl],
                in_=t_d[:, csl].rearrange("(a p) t -> p a t", p=128))

        def emit_wo_dma():
            woh = wo_p.tile([128, NPAIR, D], FP8, tag="wo", name="woh")
            wol = wo_p.tile([128, NPAIR, D], FP8, tag="wo", name="wol")
            nc.sync.dma_start(out=woh[:], in_=woh_d[:, :, :])
            nc.sync.dma_start(out=wol[:], in_=wol_d[:, :, :])
            WO["h"] = woh
            WO["l"] = wol

        def alloc_qkv(p):
            QT = qk_p.tile([128, T], BF16, tag="qk", name=f"QT{p}")
            KT = qk_p.tile([128, T], BF16, tag="qk", name=f"KT{p}")
            V = v_p.tile([128, 16, 2, VW], BF16, tag="v", name=f"V{p}")
            nc.gpsimd.memset(V[:, :, :, 64:65], WSCALE)
            QKV[p] = (QT, KT, V)

        # hi/lo 3-term schedule: (x_hi, w_hi), (x_lo, w_hi), (x_hi, w_lo)
        TERMS = (("h", "h"), ("l", "h"), ("h", "l"))

        def _qkproj_unit(p, which, c, half):
            """half of a 512-col chunk of the Q or K projection (fp8 DR)."""
            _lbl(f"proj_p{p}{which}c{c}h{half}")
            tiles = W_TILES[p]
            QT, KT, V = QKV[p]
            dst = QT if which == "q" else KT
            csl = slice(c * 512, (c + 1) * 512)
            xts = {"h": xh, "l": xl}
            if half == 0:
                ps = psum_pr.tile([128, 512], F32, tag="pr",
                                  name=f"ps{p}{which}{c}")
                PROJ_PS[(p, which, c)] = ps
                # xh-only terms first: half 0 runs before xl's DMA lands
                terms = (("h", "h"), ("h", "l"))
                first = True
            else:
                ps = PROJ_PS.pop((p, which, c))
                terms = (("l", "h"),)
                first = False
            for ti, (xs, ws) in enumerate(terms):
                w_sb = tiles[which + ws]
                x_sb = xts[xs]
                last_term = (half == 1 and ti == len(terms) - 1)
                for a in range(0, 8, 2):
                    nc.tensor.matmul(ps[:], w_sb[:, a:a + 2, :],
                                     x_sb[:, a:a + 2, csl],
                                     start=(first and ti == 0 and a == 0),
                                     stop=(last_term and a == 6),
                                     perf_mode=DR)
            if half == 1:
                nc.vector.tensor_copy(dst[:, csl], ps[:])

        def _vproj_unit(p, g, half):
            """2 t-tiles of V in [t, dv] layout (x-tile stationary, fp8)."""
            _lbl(f"vproj_p{p}g{g}h{half}")
            tiles = W_TILES[p]
            V = QKV[p][2]
            xts = {"h": xh, "l": xl}
            if half == 0:
                ps = psum_pr.tile([128, 4, 128], F32, tag="pr",
                                  name=f"psv{p}{g}")
                PROJ_PS[(p, "v", g)] = ps
            else:
                ps = PROJ_PS[(p, "v", g)]
            for j in (0, 1) if half == 0 else (2, 3):
                tt = g * 4 + j
                tsl = slice(tt * 128, (tt + 1) * 128)
                for ti, (xs, ws) in enumerate(TERMS):
                    x_sb = xts[xs]
                    w_sb = tiles["v" + ws]
                    for a in range(0, 8, 2):
                        nc.tensor.matmul(
                            ps[:, j, :], x_sb[:, a:a + 2, tsl],
                            w_sb[:, a:a + 2, :],
                            start=(ti == 0 and a == 0),
                            stop=(ti == 2 and a == 6),
                            perf_mode=DR)
            if half == 1:
                PROJ_PS.pop((p, "v", g))
                nc.vector.tensor_copy(
                    V[:, g * 4:(g + 1) * 4, :, 0:64],
                    ps[:].rearrange("p j (h d) -> p j h d", h=2))

        def qkv_units(p):
            qk = []
            vg = {g: [] for g in range(4)}
            for c in range(4):
                for half in range(2):
                    qk.append(
                        lambda p=p, c=c, h=half: _qkproj_unit(p, "q", c, h))
                for half in range(2):
                    qk.append(
                        lambda p=p, c=c, h=half: _qkproj_unit(p, "k", c, h))
                for half in range(2):
                    vg[c].append(
                        lambda p=p, c=c, h=half: _vproj_unit(p, c, h))
            return qk, vg

        class Filler:
            def __init__(self):
                self._q = []   # (tag, fn); tag = (kind, ...) or None
                self._tick = 0
            def extend(self, units, tag=None):
                self._q.extend((tag, u) for u in units)
            def push_front(self, units, tag=None):
                self._q[0:0] = [(tag, u) for u in units]
            def pop(self, every=2):
                self._tick += 1
                if self._q and self._tick % every == 0:
                    self._q.pop(0)[1]()
            def pop_kind(self, kinds):
                # pop the first queued unit of an always-ready kind
                for i, (t, _) in enumerate(self._q):
                    if t is not None and t[0] in kinds:
                        self._q.pop(i)[1]()
                        return True
                return False
            def flush(self):
                while self._q:
                    self._q.pop(0)[1]()
            def flush_kind(self, kind):
                # pop from the front until no unit of `kind` remains
                while any(t is not None and t[0] == kind
                          for t, _ in self._q):
                    self._q.pop(0)[1]()
            def force_v(self, p, g):
                # pop every queued V unit of pair p with group <= g
                i = 0
                while i < len(self._q):
                    t, _ = self._q[i]
                    if (t is not None and t[0] == "v" and t[1] == p
                            and t[2] <= g):
                        self._q.pop(i)[1]()
                    else:
                        i += 1

        def emit_qk_phase(p, qc, ctxq, filler=None, ki_lo=0, ki_hi=None,
                          ses=None, gate_ctx=None):
            QT, KT, V = QKV[p]
            nki = 4 * (qc + 1)
            qlo = qc * 512
            if ses is None:
                ses = []
            if ki_hi is None:
                ki_hi = nki
            last_pair = (p == NPAIR - 1) and SCHED_P3 in (1, 2, 3)
            for ki in range(ki_lo, min(ki_hi, nki)):
                gate = nki // 2 if SCHED_CTXI == 2 else 0
                if gate_ctx is not None and ki >= gate_ctx[0]:
                    if gate_ctx[1]:
                        gate_ctx[1].pop(0)()
                elif ctxq and ki >= gate:
                    ctxq.pop(0)()
                elif filler and last_pair:
                    # pair 3: give the normalize->transpose->hi/lo chain a
                    # head start before popping out-proj units
                    if SCHED_P3 == 3:
                        if ki >= 6:
                            filler.pop(every=1)
                    elif ki >= 5:
                        filler.pop(every=1)
                elif filler:
                    filler.pop(every=1 if p == 0 else
                               (SCHED_EVERY3 if last_pair or p == NPAIR - 1
                                else SCHED_EVERY))
                _lbl(f"sc_p{p}q{qc}k{ki}")
                ksl = slice(ki * 128, (ki + 1) * 128)
                r = ki - 4 * qc        # >= 0 -> diagonal-region ktile
                ci = max(0, r * 128)
                pss = psum_ss.tile([128, 2, 512], F32, tag="ss",
                                   name=f"pss{p}_{qc}_{ki}")
                nc.tensor.matmul(pss[:, 0, ci:], KT[0:64, ksl],
                                 QT[0:64, qlo + ci:qlo + 512],
                                 tile_position=(0, 0))
                nc.tensor.matmul(pss[:, 1, ci:], KT[64:128, ksl],
                                 QT[64:128, qlo + ci:qlo + 512],
                                 tile_position=(64, 0))
                se = se_p.tile([128, 2, 512], BF16, tag="se",
                               name=f"se{p}_{qc}_{ki}")
                nc.scalar.activation(se[:, :, ci:], pss[:, :, ci:],
                                     AF.Exp, scale=2.0 ** -13)
                if r >= 0:
                    tm = trimask[:].unsqueeze(1)
                    # final chunk: DVE is clogged with op-unit evacs right
                    # when the tail's diagonal masks gate the ctx groups;
                    # Pool is idle there
                    eng = (nc.gpsimd if (SCHED_TRIP and p == NPAIR - 1
                                         and qc == 3) else nc.vector)
                    eng.tensor_tensor(
                        out=se[:, :, ci:ci + 128], in0=se[:, :, ci:ci + 128],
                        in1=tm.broadcast_to([128, 2, 128]), op=OP.mult)
                ses.append(se)
            return ses

        def ctx_units(p, qc, ses, per_qtile=False, tsplit=False):
            # ctx accumulation: groups strictly sequential per PSUM bank
            # (start=True marks the whole 2KB bank pending-zero). qtile
            # pairs j01/j23 live in separate single-bank [128, 2, 2, VW]
            # tiles so the j01 normalize never blocks the j23 groups.
            V = QKV[p][2]
            box = {}
            def _alloc(which):
                if SCHED_CXSPLIT:
                    box[which] = psum_cx.tile(
                        [128, 2, 2, VW], F32, tag="cx" + which,
                        name=f"pctx{which}{p}_{qc}",
                        padded_shape=(128, 2, 2, 128))
                    return
                if "pctx" not in box:
                    box["pctx"] = psum_cx.tile([128, 2, 512], F32, tag="cx",
                                               name=f"pctx{p}_{qc}")
                off = 0 if which == "a" else 2 * VW
                box[which] = box["pctx"][:, :, off:off + 2 * VW].rearrange(
                    "p h (j w) -> p h j w", j=2)
            def _grp(j):
                _lbl(f"cx_p{p}q{qc}j{j}")
                which = "a" if j < 2 else "b"
                if j % 2 == 0:
                    _alloc(which)
                jsl = slice(j * 128, (j + 1) * 128)
                nk = 4 * qc + j
                for h in range(2):
                    dst = box[which][:, h, j % 2, :]
                    for ki in range(nk + 1):
                        if SCHED_GRPM:
                            # one accumulation bracket per bank: the whole
                            # tile is marked pending-zero once; each (j, h)
                            # region zeroes on its own first write, so no
                            # per-group psum sem round trips
                            st_f = (j % 2 == 0 and h == 0 and ki == 0)
                            sp_f = (j % 2 == 1 and h == 1 and ki == nk)
                        else:
                            st_f = (ki == 0)
                            sp_f = (ki == nk)
                        nc.tensor.matmul(dst, ses[ki][:, h, jsl],
                                         V[:, ki, h, :],
                                         start=st_f, stop=sp_f)
            def _grp4(j):
                # final chunk: one single-bank psum tile per qtile; j2/j3
                # borrow the score-psum ring (its exps are done by then)
                _lbl(f"cx_p{p}q{qc}j{j}")
                if j < 2:
                    tile4 = psum_cx.tile(
                        [128, 2, VW], F32, tag="cx" + ("a", "b")[j],
                        name=f"pctxF{j}", padded_shape=(128, 2, 128))
                else:
                    tile4 = psum_ss.tile([128, 2, 512], F32, tag="ss",
                                         name=f"pctxF{j}")[:, :, 0:VW]
                box[j] = tile4
                jsl = slice(j * 128, (j + 1) * 128)
                nk = 4 * qc + j
                for h in range(2):
                    dst = tile4[:, h, :]
                    for ki in range(nk + 1):
                        nc.tensor.matmul(dst, ses[ki][:, h, jsl],
                                         V[:, ki, h, :],
                                         start=(ki == 0), stop=(ki == nk))
            def _unit4(j):
                _grp4(j)
                emit_normalize_q(p, 4 * qc + j, box[j])
            if per_qtile == 4:
                return [lambda j=j: _unit4(j) for j in range(4)]
            def _unit_fp(j):
                _grp(j)
                if per_qtile and j % 2 == 1:
                    emit_normalize_half(p, qc, box["a" if j < 2 else "b"],
                                        j // 2)
                if SCHED_JPOP:
                    filler.pop(every=SCHED_EVERY)
            units = [lambda j=j: _unit_fp(j) for j in range(4)]
            if not per_qtile:
                units.append(
                    lambda: emit_normalize(p, qc, box["a"], box["b"]))
            return units

        def emit_normalize_j(p, qc, pctx, j):
            tt = 4 * qc + j
            rec = rec_p.tile([128, 2, 1], F32, tag="rec",
                             name=f"recJ{p}_{qc}_{j}")
            nc.vector.reciprocal(
                rec[:], pctx[:, :, j * VW + 64:(j + 1) * VW])
            cb = cb_p.tile([128, 2, 64], BF16, tag="cbJ",
                           name=f"cbJ{p}_{qc}_{j}")
            nc.vector.tensor_tensor(
                out=cb[:],
                in0=pctx[:, :, j * VW:j * VW + 64],
                in1=rec[:].broadcast_to([128, 2, 64]),
                op=OP.mult)
            cts = cb_p.tile([128, 128], BF16, tag="ctsJ",
                            name=f"ctsJ{p}_{qc}_{j}")
            nc.sync.dma_start_transpose(
                out=cts[:], in_=cb[:].rearrange("p h d -> p (h d)"))
            tsl = slice(tt * 128, (tt + 1) * 128)
            nc.gpsimd.tensor_copy(ctxTh[:, p, tsl], cts[:])
            nc.gpsimd.tensor_tensor(out=ctxTl[:, p, tsl],
                                    in0=cts[:],
                                    in1=ctxTh[:, p, tsl],
                                    op=OP.subtract)

        def emit_normalize_q(p, tt, tile):
            """normalize one qtile from its own [128, 2, VW] psum tile."""
            rec = rec_p.tile([128, 2, 1], F32, tag="rec", name=f"recQ{tt}")
            nc.vector.reciprocal(rec[:], tile[:, :, 64:65])
            cb = cb_p.tile([128, 2, 64], BF16, tag="cb", name=f"cbQ{tt}")
            nc.vector.tensor_tensor(
                out=cb[:], in0=tile[:, :, 0:64],
                in1=rec[:].broadcast_to([128, 2, 64]), op=OP.mult)
            cts = cb_p.tile([128, 128], BF16, tag="cts", name=f"ctsQ{tt}")
            nc.sync.dma_start_transpose(
                out=cts[:], in_=cb[:].rearrange("p h d -> p (h d)"))
            tsl = slice(tt * 128, (tt + 1) * 128)
            nc.gpsimd.tensor_copy(ctxTh[:, p, tsl], cts[:])
            nc.vector.tensor_tensor(out=ctxTl[:, p, tsl],
                                    in0=cts[:],
                                    in1=ctxTh[:, p, tsl], op=OP.subtract)

        def emit_normalize_half(p, qc, tile, half):
            """normalize one qtile-pair (a single-bank pctx tile)."""
            rec = rec_p.tile([128, 2, 2, 1], F32, tag="rec",
                             name=f"recH{p}_{qc}_{half}")
            nc.vector.reciprocal(
                rec[:], tile[:, :, :, 64:65].rearrange("p h j w -> p j h w"))
            cb = cb_p.tile([128, 2, 2, 64], BF16, tag="cb",
                           name=f"cbH{p}_{qc}_{half}")
            nc.vector.tensor_tensor(
                out=cb[:],
                in0=tile[:, :, :, 0:64].rearrange("p h j w -> p j h w"),
                in1=rec[:].broadcast_to([128, 2, 2, 64]),
                op=OP.mult)
            cts = cb_p.tile([128, 2, 128], BF16, tag="cts",
                            name=f"ctsH{p}_{qc}_{half}")
            nc.sync.dma_start_transpose(
                out=cts[:], in_=cb[:].rearrange("p j h d -> p (j h d)"))
            qsl = slice(qc * 512 + half * 256, qc * 512 + (half + 1) * 256)
            cv = cts[:].rearrange("p j q -> p (j q)")
            nc.gpsimd.tensor_copy(ctxTh[:, p, qsl], cv)
            nc.gpsimd.tensor_tensor(out=ctxTl[:, p, qsl],
                                    in0=cv,
                                    in1=ctxTh[:, p, qsl], op=OP.subtract)

        def emit_normalize(p, qc, A, B):
            rec = rec_p.tile([128, 4, 2, 1], F32, tag="rec",
                             name=f"rec{p}_{qc}")
            nc.vector.reciprocal(
                rec[:, 0:2], A[:, :, :, 64:65].rearrange("p h j w -> p j h w"))
            nc.vector.reciprocal(
                rec[:, 2:4], B[:, :, :, 64:65].rearrange("p h j w -> p j h w"))
            cb = cb_p.tile([128, 4, 2, 64], BF16, tag="cb",
                           name=f"cb{p}_{qc}")
            nc.vector.tensor_tensor(
                out=cb[:, 0:2],
                in0=A[:, :, :, 0:64].rearrange("p h j w -> p j h w"),
                in1=rec[:, 0:2].broadcast_to([128, 2, 2, 64]),
                op=OP.mult)
            nc.vector.tensor_tensor(
                out=cb[:, 2:4],
                in0=B[:, :, :, 0:64].rearrange("p h j w -> p j h w"),
                in1=rec[:, 2:4].broadcast_to([128, 2, 2, 64]),
                op=OP.mult)
            cts = cb_p.tile([128, 4, 128], BF16, tag="cts",
                            name=f"cts{p}_{qc}")
            dma_eng = (nc.scalar if (SCHED_TACT and p == NPAIR - 1
                                     and qc == 3) else nc.sync)
            dma_eng.dma_start_transpose(
                out=cts[:], in_=cb[:].rearrange("p j h d -> p (j h d)"))
            qsl = slice(qc * 512, (qc + 1) * 512)
            cv = cts[:].rearrange("p j q -> p (j q)")
            if SCHED_CHDVE and p == NPAIR - 1:
                nc.vector.tensor_copy(ctxTh[:, p, qsl], cv)
                nc.vector.tensor_tensor(out=ctxTl[:, p, qsl],
                                        in0=cv,
                                        in1=ctxTh[:, p, qsl],
                                        op=OP.subtract)
            else:
                nc.gpsimd.tensor_copy(ctxTh[:, p, qsl], cv)
                nc.gpsimd.tensor_tensor(out=ctxTl[:, p, qsl],
                                        in0=cv,
                                        in1=ctxTh[:, p, qsl],
                                        op=OP.subtract)

        def _outproj_unit(tt, ec, act_evac=False, alt=False):
            _lbl(f"op_t{tt}e{ec}")
            tsl = slice(tt * 128, (tt + 1) * 128)
            esl = slice(ec * 512, (ec + 1) * 512)
            if alt:
                # tail: score psum pool is idle; borrow a bank for ring depth
                psoT = psum_ss.tile([128, 2, 512], F32, tag="ss",
                                    name=f"psoT{tt}_{ec}")
                pso = psoT[:, 0, :]
            else:
                pso = psum_pr.tile([128, 512], F32, tag="pr",
                                   name=f"pso{tt}_{ec}")[:]
            cts = {"h": ctxTh, "l": ctxTl}
            wos = WO
            for ti, (cs, ws) in enumerate(
                    (("h", "h"), ("h", "l"), ("l", "h"))):
                ct, wo = cts[cs], wos[ws]
                for pp in (0, 2):
                    nc.tensor.matmul(pso, ct[:, pp:pp + 2, tsl],
                                     wo[:, pp:pp + 2, esl],
                                     start=(ti == 0 and pp == 0),
                                     stop=(ti == 2 and pp == 2),
                                     perf_mode=DR)
            if ec == 0:
                st = st_p.tile([128, 1024], BF16, tag="st", name=f"st{tt}")
                ST_TILES[tt] = st
            else:
                st = ST_TILES.pop(tt)
            if SCHED_LSPL and tt == 15 and ec == 1:
                # very last evac: halve across ACT and DVE in parallel so
                # the closing store fires ~0.3us earlier
                nc.scalar.activation(st[:, esl.start:esl.start + 256],
                                     pso[:, 0:256], AF.Copy)
                nc.vector.tensor_copy(st[:, esl.start + 256:esl.stop],
                                      pso[:, 256:512])
                STQ.append(lambda st=st, tsl=tsl:
                           nc.sync.dma_start(out=out_d[tsl, :], in_=st[:]))
                if not SCHED_STQ:
                    while STQ:
                        STQ.pop(0)()
                return
            if SCHED_FINEC and tt >= 16 - SCHED_FINEC:
                # closing tiles: halve the evac across ACT+DVE and store
                # each 512-col half as its own DMA
                h0 = slice(ec * 512, ec * 512 + 256)
                h1 = slice(ec * 512 + 256, ec * 512 + 512)
                nc.scalar.activation(st[:, h0], pso[:, 0:256], AF.Copy)
                nc.vector.tensor_copy(st[:, h1], pso[:, 256:512])
                STQ.append(lambda st=st, tsl=tsl, esl=esl:
                           nc.sync.dma_start(out=out_d[tsl, esl],
                                             in_=st[:, esl]))
                if not SCHED_STQ:
                    while STQ:
                        STQ.pop(0)()
                return
            if act_evac:
                nc.scalar.activation(st[:, esl], pso, AF.Copy)
            else:
                nc.vector.tensor_copy(st[:, esl], pso)
            if SCHED_ECDMA:
                STQ.append(lambda st=st, tsl=tsl, esl=esl:
                           nc.sync.dma_start(out=out_d[tsl, esl],
                                             in_=st[:, esl]))
            elif ec == 1:
                eng = nc.scalar if (SCHED_STACT and tt >= 14) else nc.sync
                STQ.append(lambda st=st, tsl=tsl, eng=eng:
                           eng.dma_start(out=out_d[tsl, :], in_=st[:]))
            if not SCHED_STQ:
                while STQ:
                    STQ.pop(0)()

        def outproj_units(qc):
            # ACT evacs on units that pop while exps still stream steal
            # throughput from the pacing engine; keep ACT evacs only where
            # SCHED_EVAC allows
            if SCHED_EVAC == 0:
                act_row = qc >= 2
            elif SCHED_EVAC == 1:
                act_row = qc >= 3
            else:
                act_row = False
            alt_row = qc >= (3 if SCHED_ALT3 else 2)
            return [lambda tt=tt, ec=ec:
                    _outproj_unit(tt, ec, act_evac=bool((tt * 2 + ec) % 2
                                  and act_row),
                                  alt=alt_row and (tt * 2 + ec) % 2 == 1)
                    for tt in range(4 * qc, 4 * (qc + 1)) for ec in range(2)]

        def emit_last_tail(ses):
            """Last chunk (pair 3, qc 3): per-qtile ctx -> normalize ->
            out-proj pipeline so the post-exp tail overlaps on all engines."""
            p, qc = NPAIR - 1, 3
            V = QKV[p][2]
            pctx = psum_cx.tile([128, 2, 512], F32, tag="cx",
                                name=f"pctx{p}_{qc}")
            def _grp(j):
                jsl = slice(j * 128, (j + 1) * 128)
                nk = 4 * qc + j
                for h in range(2):
                    dst = pctx[:, h, j * VW:(j + 1) * VW]
                    for ki in range(nk + 1):
                        nc.tensor.matmul(dst, ses[ki][:, h, jsl],
                                         V[:, ki, h, :],
                                         start=(ki == 0), stop=(ki == nk))
            def _norm(j):
                tt = 4 * qc + j
                rec = rec_p.tile([128, 2, 1], F32, tag="rec",
                                 name=f"recL{j}")
                nc.vector.reciprocal(
                    rec[:], pctx[:, :, j * VW + 64:(j + 1) * VW])
                cb = cb_p.tile([128, 2, 64], BF16, tag="cbL",
                               name=f"cbL{j}")
                nc.vector.tensor_tensor(
                    out=cb[:],
                    in0=pctx[:, :, j * VW:j * VW + 64],
                    in1=rec[:].broadcast_to([128, 2, 64]),
                    op=OP.mult)
                cts = cb_p.tile([128, 128], BF16, tag="ctsL",
                                name=f"ctsL{j}")
                nc.sync.dma_start_transpose(
                    out=cts[:], in_=cb[:].rearrange("p h d -> p (h d)"))
                tsl = slice(tt * 128, (tt + 1) * 128)
                nc.gpsimd.tensor_copy(ctxTh[:, p, tsl], cts[:])
                nc.gpsimd.tensor_tensor(out=ctxTl[:, p, tsl],
                                        in0=cts[:],
                                        in1=ctxTh[:, p, tsl],
                                        op=OP.subtract)
            # skewed pipeline: group j+1 runs on the PE while qtile j's
            # normalize chain flows through DVE/DMA/Pool
            _grp(0); _norm(0)
            _grp(1); _norm(1)
            _grp(2); _norm(2)
            _outproj_unit(12, 0, alt=False)
            _outproj_unit(12, 1, act_evac=True, alt=True)
            _grp(3); _norm(3)
            _outproj_unit(13, 0, alt=False)
            _outproj_unit(13, 1, act_evac=True, alt=True)
            _outproj_unit(14, 0, alt=False)
            _outproj_unit(14, 1, act_evac=True, alt=True)
            _outproj_unit(15, 0, alt=False)
            _outproj_unit(15, 1, act_evac=True, alt=True)

        # ---------- emission schedule ----------
        if SCHED_WARM:
            wps = psum_pr.tile([128, 512], F32, tag="pr", name="warm")
            _lbl("warm")
            for i in range(SCHED_WARM):
                nc.tensor.matmul(wps[:, 0:128], wsrc[:], wsrc[:],
                                 start=(i == 0), stop=(i == SCHED_WARM - 1))
            wsb = cb_p.tile([128, 128], BF16, tag="warm", name="warmsb")
            nc.vector.tensor_copy(wsb[:], wps[:, 0:128])
        # startup order: land the first projection's deps (wqh, xh c0) first,
        # then K weights (first score phase), V weights after the x chunks
        wt0 = {}
        for wn in ("qh",):
            w = w_p.tile([128, 8, 128], FP8, tag="w", name=f"w{wn}0")
            nc.sync.dma_start(out=w[:], in_=w_ds[wn][0])
            wt0[wn] = w
        emit_x_dma_half(0, "h")
        for wn in ("kh", "ql", "kl"):
            w = w_p.tile([128, 8, 128], FP8, tag="w", name=f"w{wn}0")
            nc.sync.dma_start(out=w[:], in_=w_ds[wn][0])
            wt0[wn] = w
        emit_x_dma_half(0, "l")
        emit_x_dma(1)
        for wn in ("vh", "vl"):
            w = w_p.tile([128, 8, 128], FP8, tag="w", name=f"w{wn}0")
            nc.sync.dma_start(out=w[:], in_=w_ds[wn][0])
            wt0[wn] = w
        W_TILES[0] = wt0
        for c in range(2, 4):
            emit_x_dma(c)
        for p in range(1, NPAIR):
            emit_w_dma(p)
        emit_wo_dma()
        for p in range(NPAIR):
            alloc_qkv(p)
        # software-pipelined chunks: emit chunk (p, qc)'s QK/exp stream with
        # the PREVIOUS chunk's ctx groups interleaved one-per-ki, so ACT
        # streams exps continuously while the PE chews older ctx matmuls.
        filler = Filler()
        u0qk, u0v = qkv_units(0)
        nup = 4 if SCHED_EARLY else len(u0qk)
        for u in u0qk[:nup]:
            u()
        filler.extend(u0qk[nup:], ("qk", 0))
        for g in range(4):
            filler.extend(u0v[g], ("v", 0, g))
        ctxq = []
        ctx_owner = (0, 0)
        heldv = []
        for p in range(NPAIR):
            last_pair = (p == NPAIR - 1)
            if not last_pair:
                qk, vg = qkv_units(p + 1)
                filler.extend(qk, ("qk", p + 1))
                for g in range(4):
                    if SCHED_HOLDV3 and p + 1 == NPAIR - 1:
                        heldv.append((vg[g], ("v", p + 1, g)))
                    else:
                        filler.extend(vg[g], ("v", p + 1, g))
            if last_pair:
                for units, tag in heldv:
                    filler.extend(units, tag)
            for qi, qc in enumerate([int(ch) for ch in SCHED_ROWS]):
                p3 = last_pair and SCHED_P3
                use_ctxq = ctxq if (SCHED_CTXI or (p3 and SCHED_P3 == 1)) \
                    else []
                # preroll: first score k-tiles go AHEAD of the ctx drain so
                # ACT has fresh psum tiles to chew while the PE drains ctx
                npre = SCHED_PREROLL if ctxq else 0
                ses = emit_qk_phase(p, qc, [], filler=None, ki_hi=npre)
                # the pending ctx must see its V tiles complete
                if ctxq:
                    filler.force_v(*ctx_owner)
                nd = 0
                while ctxq:
                    ctxq.pop(0)()
                    nd += 1
                    if SCHED_DRPOP and nd % SCHED_DRPOP == 0:
                        filler.pop_kind(("v", "qk"))
                last_chunk = last_pair and qi == 3
                lctx = None
                gate = None
                if last_chunk and SCHED_CTXL:
                    # final chunk: its own ctx groups interleave into the
                    # tail of its QK stream (group j ready at ki=13+j);
                    # per-qtile normalize cascades the ctxT chains
                    filler.force_v(p, qc)
                    lctx = ctx_units(p, qc, ses, per_qtile=SCHED_CTXLQ)
                    gate = (SCHED_CTXL, lctx)
                ses = emit_qk_phase(p, qc, use_ctxq, filler=filler,
                                    ki_lo=npre, ses=ses, gate_ctx=gate)
                while STQ:
                    STQ.pop(0)()
                if lctx is not None:
                    ctxq = lctx
                    ctx_owner = (p, qc)
                    ctxq.append(
                        lambda qc=qc: filler.extend(outproj_units(qc),
                                                    ("op", qc)))
                elif (p3 or SCHED_P3 == 4) and qc == 3 and p == NPAIR - 1:
                    emit_last_tail(ses)
                    ctxq = []
                else:
                    ctxq = ctx_units(p, qc, ses,
                                     per_qtile=bool(SCHED_P3Q and last_pair),
                                     tsplit=bool(SCHED_TSPLIT and last_pair
                                                 and qi == 3))
                    ctx_owner = (p, qc)
                    if last_pair:
                        ctxq.append(
                            lambda qc=qc: filler.extend(outproj_units(qc),
                                                        ("op", qc)))
            if not last_pair:
                # only the next pair's Q/K must exist before its QK phases;
                # V units stay queued as fillers for the next pair's window
                filler.flush_kind("qk")
        while ctxq:
            ctxq.pop(0)()
        filler.flush()
        while STQ:
            STQ.pop(0)()

    nc.compile()
    _nc_cache = nc
    return nc


def kernel(x, Wq, Wk, Wv, Wo):
    import ml_dtypes

    from concourse.bass_utils import run_bass_kernel_spmd

    F8 = ml_dtypes.float8_e4m3fn
    nc = _build()
    x = np.asarray(x, dtype=np.float32)
    Wq, Wk, Wv, Wo = (np.asarray(w, dtype=np.float32)
                      for w in (Wq, Wk, Wv, Wo))

    def hilo(a):
        hi = a.astype(F8)
        lo = (a - hi.astype(np.float32)).astype(F8)
        return hi, lo

    def pack_w(Wt):
        # 32*Wt: [sl-feat, 1024] -> hi/lo tiles [pair, 128(d%128), 8, 128]
        t = (WSCALE * Wt.T).reshape(8, 128, NPAIR, 128)  # [a, pd, pair, f]
        t = np.ascontiguousarray(t.transpose(2, 1, 0, 3))
        return hilo(t)

    in_maps = []
    for c in range(NCORES):
        b, hg = c // 2, c % 2
        sl = slice(hg * ESH, (hg + 1) * ESH)
        xth, xtl = hilo(np.ascontiguousarray(x[b].T))
        wqh, wql = pack_w(Wq[sl, :])
        wkh, wkl = pack_w(Wk[sl, :])
        wvh, wvl = pack_w(Wv[sl, :])
        wall = np.ascontiguousarray(
            np.stack([wqh, wql, wkh, wkl, wvh, wvl], axis=1))
        wot = (WSCALE * Wo[:, sl].T).reshape(NPAIR, 128, D)
        woh, wol = hilo(np.ascontiguousarray(wot.transpose(1, 0, 2)))
        in_maps.append({
            "xh": xth, "xl": xtl, "wall": wall,
            "wqh": wqh, "wql": wql, "wkh": wkh, "wkl": wkl,
            "wvh": wvh, "wvl": wvl,
            "woh": woh, "wol": wol,
        })
    res = run_bass_kernel_spmd(nc, in_maps, list(range(NCORES)))
    outs = [np.asarray(res.results[c]["out"]).astype(np.float32)
            for c in range(NCORES)]
    return np.stack([(outs[2 * b] + outs[2 * b + 1]) * (1.0 / WSCALE)
                     for b in range(B)])

